# revision 13
# baseline (speedup 1.0000x reference)
"""RGCN 2-layer kernel for Trainium2, 8 NeuronCores.

Strategy (dst-node sharding):
  - Each core owns 4096 dst nodes; edges are routed to the core owning dst.
  - Host packs each core's edges (sorted by dst) into "windows": <=16 dst
    nodes and <=256 edge slots per window (2 chunks of 128).
  - Aggregation = one-hot matmul: gathered message chunk [128 edges, 256 f]
    (bf16, via dma_gather from the AllGather'd node table in DRAM) is the
    stationary operand; A-chunk [128 edges, 4b x 16 slots] (host-built,
    value = comp[rel,b]/cnt(rel,dst)) streams -> psum [128 f-half, slots].
    This yields Mb^T (basis-projected means, feature-major) directly.
  - Output matmul: out^T[o, slot] = sum_b basis_b^T @ Mb^T + root^T @ h^T
    accumulated in PSUM; +bias (+relu for layer 0) on ACT engine.
  - h^T slot columns are transposed back to rows on PE and scattered
    (dma_scatter_add into a zeroed table) to node-indexed DRAM rows.
  - AllGather shares the x shards (input) and the layer-1 input table.

Host/runner strategy (the axon tunnel moves ~50MB/s, so warm-call cost is
dominated by transferred bytes and per-call jit rebuild):
  - The jitted shard_map(bass_exec) callable is built ONCE and cached.
  - All graph-derived data (gather/scatter indices, A matrices, weights)
    lives on device across calls; a sha256 of the non-x inputs validates it.
  - Per call only the x shards (16MB bf16 total) go up and the output
    (f32) comes back.
"""

import os
import functools
import hashlib
from concurrent.futures import ThreadPoolExecutor
import numpy as np
import ml_dtypes

import jax
import jax.numpy as jnp
from jax.sharding import Mesh, PartitionSpec, NamedSharding
from jax.experimental.shard_map import shard_map

import concourse.bass as bass
import concourse.mybir as mybir
from concourse import bacc
from concourse import tile
from concourse.bass2jax import (
    _bass_exec_p,
    fast_dispatch_compile,
    install_neuronx_cc_hook,
    partition_id_tensor,
)

N_CORES = 8
NODES = 32768
NPC = 4096              # nodes per core
D = 256
R, NB = 8, 4
WIN_EDGES = 256         # edge slots per window (2 chunks of 128)
WIN_SLOTS = 16          # node slots per window
WINB = 32               # windows per batch
BATCH_SLOTS = WINB * WIN_SLOTS    # 512
BATCH_EDGES = WINB * WIN_EDGES    # 8192
BATCH_CHUNKS = BATCH_EDGES // 128  # 64

BF16 = ml_dtypes.bfloat16
F32 = np.float32

_PROG_CACHE = {}                 # nbatch -> (nc, runner)
_STATIC = {"key": None, "nbatch": None, "dev": None}
DBG_LAYERS = int(os.environ.get("KDBG_LAYERS", "2"))     # 1 = layer 0 only
DBG_COLL = int(os.environ.get("KDBG_COLL", "1"))         # 0 = skip collective
DBG_PHASE = int(os.environ.get("KDBG_PHASE", "4"))       # 1..4 pipeline depth

STATIC_NAMES = ("edge_cond", "relation_cond",
                "basis0", "comp0", "root0", "bias0",
                "basis1", "comp1", "root1", "bias1")


def _pack_core(dst_loc, n_nodes):
    """Pack nodes (sorted) into windows of <=WIN_SLOTS nodes / <=WIN_EDGES edges.

    Returns (windows, node_slot) where windows is a list of
    (node_list, edge_start_count_list); edges for node n are the contiguous
    run in the dst-sorted order.
    """
    cnt = np.bincount(dst_loc, minlength=n_nodes)
    starts = np.concatenate([[0], np.cumsum(cnt)[:-1]])
    windows = []
    cur_nodes = []
    cur_edges = 0
    for n in range(n_nodes):
        c = int(cnt[n])
        if len(cur_nodes) >= WIN_SLOTS or cur_edges + c > WIN_EDGES:
            windows.append(cur_nodes)
            cur_nodes = []
            cur_edges = 0
        cur_nodes.append(n)
        cur_edges += c
    if cur_nodes:
        windows.append(cur_nodes)
    return windows, cnt, starts


def _build_core_data(src, dst, rel, core, w_edge, comp0, comp1, nbatch):
    """Host-side per-core packing (vectorized). Returns dict of device arrays."""
    lo, hi = core * NPC, (core + 1) * NPC
    m = (dst >= lo) & (dst < hi)
    e_src = src[m]
    e_dst = dst[m] - lo
    e_rel = rel[m]
    e_w = w_edge[m]
    order = np.argsort(e_dst, kind="stable")
    e_src, e_dst, e_rel, e_w = e_src[order], e_dst[order], e_rel[order], e_w[order]

    windows, cnt, starts = _pack_core(e_dst, NPC)
    nw = nbatch * WINB
    assert len(windows) <= nw, (len(windows), nw)

    ne = nw * WIN_EDGES
    ns = nw * WIN_SLOTS

    # per-node window/slot arrays
    win_of = np.empty(NPC, np.int64)
    slot_of = np.empty(NPC, np.int64)
    first_of_win = np.empty(len(windows), np.int64)
    for w, nodes in enumerate(windows):
        nn = np.asarray(nodes)
        win_of[nn] = w
        slot_of[nn] = np.arange(len(nn))
        first_of_win[w] = nodes[0]

    # edge-slot base per node: cumulative count within its window
    csum = np.concatenate([[0], np.cumsum(cnt)])          # csum[n] = edges before node n
    node_ebase = csum[:-1] - csum[first_of_win[win_of]]   # within-window offset
    # per-edge slot position
    ranks = np.arange(len(e_dst)) - starts[e_dst]         # rank within node run
    pos = win_of[e_dst] * WIN_EDGES + node_ebase[e_dst] + ranks

    gidx = np.zeros(ne, np.int16)
    gidx[pos] = e_src.astype(np.int16)
    sgidx = np.zeros(ns, np.int16)
    scidx = np.full(ns, NPC, np.int16)
    all_nodes = np.arange(NPC)
    spos = win_of * WIN_SLOTS + slot_of
    sgidx[spos] = (lo + all_nodes).astype(np.int16)
    scidx[spos] = all_nodes.astype(np.int16)

    a0 = np.zeros((ne, NB * WIN_SLOTS), F32)
    a1 = np.zeros((ne, NB * WIN_SLOTS), F32)
    esl = slot_of[e_dst]
    c0 = comp0.astype(F32)
    c1 = comp1.astype(F32)
    for b in range(NB):
        a0[pos, b * WIN_SLOTS + esl] = e_w * c0[e_rel, b]
        a1[pos, b * WIN_SLOTS + esl] = e_w * c1[e_rel, b]

    # wrap indices to [128, n/16]: element [16k+p, s] = arr[s*16 + p]
    # (16-partition wrap, replicated 8x so each Q7 core sees the data)
    def wrap(a):
        return np.ascontiguousarray(np.tile(a.reshape(-1, 16).T, (8, 1)))

    # A device layout: [2, nbatch, 128, BATCH_CHUNKS*64]
    def alay(a):
        a = a.reshape(nbatch, BATCH_CHUNKS, 128, NB * WIN_SLOTS)
        a = a.transpose(0, 2, 1, 3).reshape(nbatch, 128, BATCH_CHUNKS * NB * WIN_SLOTS)
        return a.astype(BF16)

    return {
        "gidx": wrap(gidx),
        "sgidx": wrap(sgidx),
        "scidx": wrap(scidx),
        "A": np.stack([alay(a0), alay(a1)]),
    }


def _build_program(nbatch):
    dt = mybir.dt
    ns_total = nbatch * WINB * WIN_SLOTS
    ne_total = nbatch * WINB * WIN_EDGES

    nc = bacc.Bacc("TRN2", target_bir_lowering=False, debug=False,
                   num_devices=N_CORES)

    xs_d = nc.dram_tensor("x_shard", [NPC, D], dt.bfloat16, kind="ExternalInput")
    gidx_d = nc.dram_tensor("gidx", [128, ne_total // 16], dt.int16, kind="ExternalInput")
    sgidx_d = nc.dram_tensor("sgidx", [128, ns_total // 16], dt.int16, kind="ExternalInput")
    scidx_d = nc.dram_tensor("scidx", [128, ns_total // 16], dt.int16, kind="ExternalInput")
    a_d = nc.dram_tensor("A", [2, nbatch, 128, BATCH_CHUNKS * NB * WIN_SLOTS],
                         dt.bfloat16, kind="ExternalInput")
    basis_d = nc.dram_tensor("basis_sb", [128, 2 * NB * 2 * 2 * 128], dt.bfloat16,
                             kind="ExternalInput")
    root_d = nc.dram_tensor("root_sb", [128, 2 * 2 * 2 * 128], dt.bfloat16,
                            kind="ExternalInput")
    ident_d = nc.dram_tensor("ident_bf", [128, 128], dt.bfloat16, kind="ExternalInput")
    identf_d = nc.dram_tensor("ident_f32", [128, 128], dt.float32, kind="ExternalInput")
    bias_d = nc.dram_tensor("bias_sb", [128, 4], dt.float32, kind="ExternalInput")
    zeros_d = nc.dram_tensor("zeros_t", [NPC + 1, D], dt.bfloat16, kind="ExternalInput")

    x_loc = nc.dram_tensor("x_loc", [NPC, D], dt.bfloat16)
    xt = nc.dram_tensor("x_full", [NODES, D], dt.bfloat16, addr_space="Shared")
    h1_loc = nc.dram_tensor("h1_loc", [NPC + 1, D], dt.bfloat16)
    h1_full = nc.dram_tensor("h1_full", [NODES, D], dt.bfloat16, addr_space="Shared")
    out_d = nc.dram_tensor("out", [NPC + 1, D], dt.bfloat16, kind="ExternalOutput")

    AF = mybir.ActivationFunctionType

    with tile.TileContext(nc) as tc:
        with (
            tc.tile_pool(name="const", bufs=1) as constp,
            tc.tile_pool(name="gath", bufs=3) as gathp,
            tc.tile_pool(name="abuf", bufs=3) as abufp,
            tc.tile_pool(name="mbt", bufs=2) as mbtp,
            tc.tile_pool(name="x0t", bufs=2) as x0tp,
            tc.tile_pool(name="h1t", bufs=1) as h1tp,
            tc.tile_pool(name="outt", bufs=2) as outtp,
            tc.tile_pool(name="rows", bufs=2) as rowsp,
            tc.tile_pool(name="rowsf", bufs=2) as rowsfp,
            tc.tile_pool(name="ps_a", bufs=2, space="PSUM") as psa,
            tc.tile_pool(name="ps_o", bufs=2, space="PSUM") as pso,
            tc.tile_pool(name="ps_t", bufs=2, space="PSUM") as pst,
        ):
            # ---- share the input shards: x_full = AllGather(x_shard) ----
            # (collectives cannot read IO tensors; stage through x_loc)
            if DBG_COLL:
                nc.sync.dma_start(out=x_loc[:, :], in_=xs_d[:, :])
                nc.gpsimd.collective_compute(
                    "AllGather", mybir.AluOpType.bypass,
                    replica_groups=[list(range(N_CORES))],
                    ins=[x_loc[:, :]],
                    outs=[xt[:, :]],
                )

            # ---- constants ----
            basis_sb = constp.tile([128, 2 * NB * 2 * 2 * 128], dt.bfloat16)
            nc.sync.dma_start(out=basis_sb[:], in_=basis_d[:, :])
            root_sb = constp.tile([128, 2 * 2 * 2 * 128], dt.bfloat16)
            nc.sync.dma_start(out=root_sb[:], in_=root_d[:, :])
            ident = constp.tile([128, 128], dt.bfloat16)
            nc.sync.dma_start(out=ident[:], in_=ident_d[:, :])
            identf = constp.tile([128, 128], dt.float32)
            nc.sync.dma_start(out=identf[:], in_=identf_d[:, :])
            bias_sb = constp.tile([128, 4], dt.float32)
            nc.sync.dma_start(out=bias_sb[:], in_=bias_d[:, :])
            gidx_sb = constp.tile([128, ne_total // 16], dt.int16)
            nc.sync.dma_start(out=gidx_sb[:], in_=gidx_d[:, :])
            sgidx_sb = constp.tile([128, ns_total // 16], dt.int16)
            nc.sync.dma_start(out=sgidx_sb[:], in_=sgidx_d[:, :])
            scidx_sb = constp.tile([128, ns_total // 16], dt.int16)
            nc.sync.dma_start(out=scidx_sb[:], in_=scidx_d[:, :])

            # resident h1^T slot columns (root rhs for layer 1)
            h1t_slots = h1tp.tile([128, 2, ns_total], dt.bfloat16)

            # pre-zero the local h1 table and the output (scatter_add accumulates;
            # zeroing out_d in-NEFF means the runner needn't donate zero buffers)
            nc.sync.dma_start(out=h1_loc[:, :], in_=zeros_d[:, :])
            nc.sync.dma_start(out=out_d[:, :], in_=zeros_d[:, :])

            for layer in range(DBG_LAYERS):
                table = xt if layer == 0 else h1_full
                for bt in range(nbatch):
                    # gather messages for this batch: [128, chunks, 256] bf16
                    gbuf = gathp.tile([128, BATCH_CHUNKS, D], dt.bfloat16)
                    # single_packet=False is required above ~1024 idxs/call;
                    # one big call per batch avoids per-call SWDGE overhead
                    nc.gpsimd.dma_gather(
                        gbuf[:], table[:, :],
                        gidx_sb[:, bt * (BATCH_EDGES // 16):(bt + 1) * (BATCH_EDGES // 16)],
                        BATCH_EDGES, BATCH_EDGES, D, single_packet=False,
                    )
                    abuf = abufp.tile([128, BATCH_CHUNKS * NB * WIN_SLOTS], dt.bfloat16)
                    nc.sync.dma_start(out=abuf[:], in_=a_d[layer, bt, :, :])

                    if layer == 0:
                        # root rhs: x^T columns in slot order via transposed gather
                        x0t = x0tp.tile([128, 2, BATCH_SLOTS], dt.bfloat16)
                        nc.gpsimd.dma_gather(
                            x0t[:], xt[:, :],
                            sgidx_sb[:, bt * (BATCH_SLOTS // 16):(bt + 1) * (BATCH_SLOTS // 16)],
                            BATCH_SLOTS, BATCH_SLOTS, D, transpose=True,
                        )

                    # aggregation: Mb^T for this batch, [128, fhalf, b, slots]
                    mbt = mbtp.tile([128, 2, NB, BATCH_SLOTS], dt.bfloat16)
                    for g in range(WINB // 4 if DBG_PHASE >= 2 else 0):  # 4-window psum groups
                        ps0 = psa.tile([128, 4 * NB * WIN_SLOTS], dt.float32, tag="psA")
                        ps1 = psa.tile([128, 4 * NB * WIN_SLOTS], dt.float32, tag="psB")
                        for wl in range(4):
                            w = g * 4 + wl
                            for ch in range(2):
                                c = w * 2 + ch
                                rhs = abuf[:, c * 64:(c + 1) * 64]
                                nc.tensor.matmul(
                                    ps0[:, wl * 64:(wl + 1) * 64],
                                    gbuf[:, c, 0:128], rhs,
                                    start=(ch == 0), stop=(ch == 1),
                                )
                                nc.tensor.matmul(
                                    ps1[:, wl * 64:(wl + 1) * 64],
                                    gbuf[:, c, 128:256], rhs,
                                    start=(ch == 0), stop=(ch == 1),
                                )
                        # flush psum (w,b,s) -> mbt[:, half, b, g*64 + (w,s)]
                        ps0v = ps0[:].rearrange("p (w b s) -> p w b s", w=4, b=NB, s=WIN_SLOTS)
                        ps1v = ps1[:].rearrange("p (w b s) -> p w b s", w=4, b=NB, s=WIN_SLOTS)
                        for b in range(NB):
                            dst0 = mbt[:, 0, b, g * 64:(g + 1) * 64]
                            dst1 = mbt[:, 1, b, g * 64:(g + 1) * 64]
                            nc.vector.tensor_copy(
                                dst0.rearrange("p (w s) -> p w s", w=4), ps0v[:, :, b, :])
                            nc.vector.tensor_copy(
                                dst1.rearrange("p (w s) -> p w s", w=4), ps1v[:, :, b, :])

                    # output matmuls: out^T[o, slot] accumulated over (b, ih) + root
                    if layer == 1:
                        outt = outtp.tile([128, 2, BATCH_SLOTS], dt.float32)
                    for oh in range(2 if DBG_PHASE >= 3 else 0):
                        po = pso.tile([128, BATCH_SLOTS], dt.float32, tag="psO")
                        k = 0
                        for b in range(NB):
                            for ih in range(2):
                                wof = (((layer * NB + b) * 2 + ih) * 2 + oh) * 128
                                nc.tensor.matmul(
                                    po[:], basis_sb[:, wof:wof + 128],
                                    mbt[:, ih, b, :],
                                    start=(k == 0), stop=False)
                                k += 1
                        for ih in range(2):
                            wof = ((layer * 2 + ih) * 2 + oh) * 128
                            rrhs = (x0t[:, ih, :] if layer == 0
                                    else h1t_slots[:, ih, bt * BATCH_SLOTS:(bt + 1) * BATCH_SLOTS])
                            nc.tensor.matmul(
                                po[:], root_sb[:, wof:wof + 128], rrhs,
                                start=False, stop=(ih == 1))
                        if layer == 0:
                            nc.scalar.activation(
                                h1t_slots[:, oh, bt * BATCH_SLOTS:(bt + 1) * BATCH_SLOTS],
                                po[:], AF.Relu, bias=bias_sb[:, 0 + oh:1 + oh])
                        else:
                            nc.scalar.activation(
                                outt[:, oh, :], po[:], AF.Identity,
                                bias=bias_sb[:, 2 + oh:3 + oh])

                    # transpose slot columns back to rows and scatter to DRAM
                    if DBG_PHASE < 4:
                        continue
                    if layer == 0:
                        rows = rowsp.tile([128, 4, D], dt.bfloat16)
                        for sb4 in range(4):
                            for fh in range(2):
                                pt = pst.tile([128, 128], dt.bfloat16, tag="psT")
                                src = h1t_slots[:, fh,
                                                bt * BATCH_SLOTS + sb4 * 128:
                                                bt * BATCH_SLOTS + (sb4 + 1) * 128]
                                nc.tensor.transpose(pt[:], src, ident[:])
                                nc.scalar.activation(
                                    rows[:, sb4, fh * 128:(fh + 1) * 128], pt[:], AF.Copy)
                        nc.gpsimd.dma_scatter_add(
                            h1_loc[:, :], rows[:],
                            scidx_sb[:, bt * (BATCH_SLOTS // 16):(bt + 1) * (BATCH_SLOTS // 16)],
                            BATCH_SLOTS, BATCH_SLOTS, D)
                    else:
                        rowsf = rowsfp.tile([128, 4, D], dt.bfloat16)
                        for sb4 in range(4):
                            for fh in range(2):
                                pt = pst.tile([128, 128], dt.float32, tag="psT")
                                nc.tensor.transpose(
                                    pt[:], outt[:, fh, sb4 * 128:(sb4 + 1) * 128], identf[:])
                                nc.scalar.activation(
                                    rowsf[:, sb4, fh * 128:(fh + 1) * 128], pt[:], AF.Copy)
                        nc.gpsimd.dma_scatter_add(
                            out_d[:, :], rowsf[:],
                            scidx_sb[:, bt * (BATCH_SLOTS // 16):(bt + 1) * (BATCH_SLOTS // 16)],
                            BATCH_SLOTS, BATCH_SLOTS, D)

                if layer == 0 and DBG_COLL:
                    nc.gpsimd.collective_compute(
                        "AllGather", mybir.AluOpType.bypass,
                        replica_groups=[list(range(N_CORES))],
                        ins=[h1_loc[0:NPC, :]],
                        outs=[h1_full[:, :]],
                    )

    nc.compile()
    return nc


def _make_runner(nc):
    """Build the once-per-program jitted shard_map callable + metadata."""
    install_neuronx_cc_hook()
    partition_name = (nc.partition_id_tensor.name
                      if nc.partition_id_tensor is not None else None)
    in_names, out_names, out_avals, zero_shapes = [], [], [], []
    for alloc in nc.m.functions[0].allocations:
        if not isinstance(alloc, mybir.MemoryLocationSet):
            continue
        name = alloc.memorylocations[0].name
        if alloc.kind == "ExternalInput":
            if name != partition_name:
                in_names.append(name)
        elif alloc.kind == "ExternalOutput":
            out_names.append(name)
            shape = tuple(alloc.tensor_shape)
            dtype = mybir.dt.np(alloc.dtype)
            out_avals.append(jax.core.ShapedArray(shape, dtype))
            zero_shapes.append((shape, dtype))
    n_params = len(in_names)
    n_outs = len(out_names)
    all_in_names = list(in_names) + list(out_names)
    if partition_name:
        all_in_names.append(partition_name)

    def _body(*args):
        operands = list(args)
        if partition_name:
            operands.append(partition_id_tensor())
        outs = _bass_exec_p.bind(
            *operands, out_avals=tuple(out_avals),
            in_names=tuple(all_in_names), out_names=tuple(out_names),
            lowering_input_output_aliases=(), sim_require_finite=True,
            sim_require_nnan=True, nc=nc)
        return tuple(outs)

    devices = jax.devices()[:N_CORES]
    assert len(devices) == N_CORES
    mesh = Mesh(np.asarray(devices), ("core",))
    in_specs = (PartitionSpec("core"),) * (n_params + n_outs)
    out_specs = (PartitionSpec("core"),) * n_outs
    sh = NamedSharding(mesh, PartitionSpec("core"))

    # dtype lookup for the per-core input avals
    in_dtypes = {}
    for alloc in nc.m.functions[0].allocations:
        if isinstance(alloc, mybir.MemoryLocationSet) and alloc.kind == "ExternalInput":
            in_dtypes[alloc.memorylocations[0].name] = (
                tuple(alloc.tensor_shape), mybir.dt.np(alloc.dtype))

    arg_sds = []
    for name in in_names:
        shape, d = in_dtypes[name]
        arg_sds.append(jax.ShapeDtypeStruct(
            (N_CORES * shape[0],) + tuple(shape[1:]), d, sharding=sh))
    for (s, d) in zero_shapes:
        arg_sds.append(jax.ShapeDtypeStruct(
            (N_CORES * s[0],) + tuple(s[1:]), d, sharding=sh))

    # out_d is zeroed inside the NEFF, so the "out" operand is never read:
    # no donation -> the same persistent dummy buffer can be passed per call.
    # fast_dispatch_compile suppresses the bass effect -> C++ fast dispatch.
    def _compile():
        jitted = jax.jit(shard_map(_body, mesh=mesh, in_specs=in_specs,
                                   out_specs=out_specs, check_rep=False),
                         keep_unused=True)
        return jitted.lower(*arg_sds).compile()

    fn = fast_dispatch_compile(_compile)
    out_dummies = [
        jax.jit(functools.partial(jnp.zeros, (N_CORES * s[0],) + tuple(s[1:]), d),
                out_shardings=sh)()
        for (s, d) in zero_shapes
    ]
    return {"fn": fn, "in_names": in_names, "out_names": out_names,
            "out_dummies": out_dummies, "sh": sh, "mesh": mesh}


def _static_key(inputs):
    h = hashlib.sha256()
    for name in STATIC_NAMES:
        a = np.ascontiguousarray(np.asarray(inputs[name]))
        h.update(name.encode())
        h.update(str(a.dtype).encode())
        h.update(a.tobytes())
    return h.hexdigest()


def _cold_build(inputs, key):
    edge = np.asarray(inputs["edge_cond"]).astype(np.int64)
    relc = np.asarray(inputs["relation_cond"]).astype(np.int64)
    basis0 = np.asarray(inputs["basis0"], F32)
    comp0 = np.asarray(inputs["comp0"], F32)
    root0 = np.asarray(inputs["root0"], F32)
    bias0 = np.asarray(inputs["bias0"], F32)
    basis1 = np.asarray(inputs["basis1"], F32)
    comp1 = np.asarray(inputs["comp1"], F32)
    root1 = np.asarray(inputs["root1"], F32)
    bias1 = np.asarray(inputs["bias1"], F32)

    src = edge[0]
    dst = edge[1]
    rel = relc

    # per-(rel, dst) counts -> per-edge mean weights
    seg = rel * NODES + dst
    cnt = np.bincount(seg, minlength=R * NODES)
    w_edge = (1.0 / np.maximum(cnt[seg], 1)).astype(F32)

    # figure out the uniform batch count across cores
    max_w = 0
    for c in range(N_CORES):
        m = (dst >= c * NPC) & (dst < (c + 1) * NPC)
        d_loc = np.sort(dst[m] - c * NPC)
        windows, _, _ = _pack_core(d_loc, NPC)
        max_w = max(max_w, len(windows))
    nbatch = (max_w + WINB - 1) // WINB

    if nbatch not in _PROG_CACHE:
        nc = _build_program(nbatch)
        runner = _make_runner(nc)
        _PROG_CACHE[nbatch] = (nc, runner)
    nc, runner = _PROG_CACHE[nbatch]

    # weights in stationary layout [128, ...] bf16
    def wlay(mat):  # [256, 256] -> [128, 2, 2, 128] (i_in_half, ih, oh, o)
        m4 = mat.reshape(2, 128, 2, 128)        # [ih, i, oh, o]
        return np.ascontiguousarray(m4.transpose(1, 0, 2, 3)).astype(BF16)

    basis_sb = np.zeros((128, 2, NB, 2, 2, 128), BF16)
    for b in range(NB):
        basis_sb[:, 0, b] = wlay(basis0[b])
        basis_sb[:, 1, b] = wlay(basis1[b])
    basis_sb = basis_sb.reshape(128, -1)
    root_sb = np.stack([wlay(root0), wlay(root1)], axis=1).reshape(128, -1)
    ident = np.eye(128, dtype=BF16)
    identf = np.eye(128, dtype=F32)
    bias_sb = np.stack(
        [bias0[:128], bias0[128:], bias1[:128], bias1[128:]], axis=1
    ).astype(F32)
    zeros_t = np.zeros((NPC + 1, D), BF16)

    shared = {"basis_sb": basis_sb, "root_sb": root_sb, "ident_bf": ident,
              "ident_f32": identf, "bias_sb": bias_sb, "zeros_t": zeros_t}

    core_data = [_build_core_data(src, dst, rel, c, w_edge, comp0, comp1, nbatch)
                 for c in range(N_CORES)]

    sh = runner["sh"]
    dev = {}
    for name in runner["in_names"]:
        if name == "x_shard":
            continue
        if name in shared:
            arr = np.concatenate([shared[name]] * N_CORES, axis=0)
        else:
            arr = np.concatenate([cd[name] for cd in core_data], axis=0)
        dev[name] = jax.device_put(arr, sh)
    for v in dev.values():
        v.block_until_ready()

    _STATIC["key"] = key
    _STATIC["nbatch"] = nbatch
    _STATIC["dev"] = dev


def kernel(**inputs):
    key = _static_key(inputs)
    if _STATIC["key"] != key:
        _cold_build(inputs, key)

    nc, runner = _PROG_CACHE[_STATIC["nbatch"]]
    dev = _STATIC["dev"]

    x = np.asarray(inputs["x"])
    xb = np.ascontiguousarray(x.reshape(NODES, D)).astype(BF16)
    xdev = jax.device_put(xb, runner["sh"])

    args = [xdev if n == "x_shard" else dev[n] for n in runner["in_names"]]
    outs = runner["fn"](*args, *runner["out_dummies"])
    out = np.asarray(outs[0])                        # [8*(NPC+1), D] bf16
    out = out.reshape(N_CORES, NPC + 1, D)[:, :NPC, :]
    res = np.empty((N_CORES, NPC, D), F32)

    def _conv(c):
        res[c] = out[c]
    with ThreadPoolExecutor(N_CORES) as pool:
        list(pool.map(_conv, range(N_CORES)))
    return res
